# revision 28
# baseline (speedup 1.0000x reference)
"""Causal multi-head attention block on 8 TRN2 NeuronCores — v3.

Sharding: tensor-parallel over heads (2 heads/core, both batches) for the
QKV projection + attention; on-device AllToAlls re-shard to
sequence-parallel for the output projection (Megatron-style).

v3 restructure vs v2:
- A2A split by BATCH, not q-half: core c owns out rows 256c..256c+256 of
  batch 0 and rows 2048+256c.. of batch 1. The batch-0 AllToAll launches
  right after batch-0 attention finishes and is fully hidden under
  batch-1 attention; batch-0's output projection runs as paced PE filler
  inside the batch-1 attention stream. Only the batch-1 A2A + its
  projection remain in the tail (and the first two tail pos-groups are
  batch-0 work that keeps PE warm while the collective flies).
- yts DMAs ride the gpsimd queue (ordered after the collective, so no
  compute engine ever blocks on the collective semaphore).
- projection filler pacing is piecewise-linear against hard per-chunk
  deadlines instead of burst-at-chunk-start.
- diagonal score matmuls truncate their column range to >=256 (fp32r
  keeps full rate at N>=256).

Self-contained: hardcodes all shapes from the problem spec.
"""

import numpy as np
from contextlib import ExitStack

import concourse.bass as bass
import concourse.tile as tile
from concourse import bacc, mybir
from concourse.bass_utils import run_bass_kernel_spmd

F32R = mybir.dt.float32r
F32 = mybir.dt.float32
BF16 = mybir.dt.bfloat16
AF = mybir.ActivationFunctionType

B, T, C, H, HD = 2, 2048, 1024, 16, 64
NCORES = 8
BT = B * T            # 4096 global rows
TQ = 512              # q-chunk width
KT = 128              # k-tile height
NJ = T // TQ          # 4 q-chunks per batch (= per core)
NKK = T // KT         # 16 k-tiles per batch
NCT = C // 128        # 8 contraction tiles for projections
NTC = BT // TQ        # 8 global t-chunks
TSL = BT // NCORES    # 512 rows of final output per core
HTQ = TQ // 2         # 256: unit width for the batch A2A
XT_SHAPE = [NTC, 128, NCT, TQ]  # t-chunk major, partition-major inside

# proj-filler deadlines: before emitting global attention step s, at least
# F(s) filler steps must be done (5 steps per chunk; chunk order
# (0,1),(0,2),(0,3),(1,0),(1,1),(1,2),(1,3)); linear ramp between points.
_FILL_HARD = [(0, 0), (4, 5), (12, 10), (24, 15), (40, 20), (44, 25),
              (52, 30), (64, 35), (80, 35)]


def _fill_req(s):
    for (s0, f0), (s1, f1) in zip(_FILL_HARD, _FILL_HARD[1:]):
        if s0 <= s <= s1:
            if s1 == s0:
                return f1
            import math
            return int(math.ceil(f0 + (f1 - f0) * (s - s0) / (s1 - s0)))
    return 35


def build(with_collective=True):
    nc = bacc.Bacc(None, target_bir_lowering=False)

    xt = nc.dram_tensor("xt", XT_SHAPE, BF16, kind="ExternalInput")
    wqkv = nc.dram_tensor("wqkv", [C, 3 * 128], BF16, kind="ExternalInput")
    bqkv = nc.dram_tensor("bqkv", [128, 3], F32, kind="ExternalInput")
    wout = nc.dram_tensor("wout", [C, C], BF16, kind="ExternalInput")
    bout = nc.dram_tensor("bout", [128, C], F32, kind="ExternalInput")
    out = nc.dram_tensor("out", [TSL, C], F32, kind="ExternalOutput")

    ident_d = nc.dram_tensor("ident", [128, 128], BF16, kind="ExternalInput")
    mskw_d = nc.dram_tensor("mskw", [128, 128], BF16, kind="ExternalInput")
    # batch-major: a2a_in[p, d] = this core's heads of batch-p unit d
    # (unit d = q-cols 256*(d%2).. of q-chunk d//2); slot d -> core d.
    a2a_in = nc.dram_tensor("a2a_in", [2, NCORES, 128, HTQ], BF16)
    a2a_out = nc.dram_tensor("a2a_out", [2, NCORES, 128, HTQ], BF16)

    with tile.TileContext(nc) as tc:
        _emit(nc, tc, xt, wqkv, bqkv, wout, bout, out, a2a_in, a2a_out,
              ident_d, mskw_d, with_collective)
    nc.compile()
    return nc


def _emit(nc, tc, xt, wqkv, bqkv, wout, bout, out, a2a_in, a2a_out,
          ident_d, mskw_d, with_collective, trunc=None):
    with ExitStack() as ctx:
        persist = ctx.enter_context(tc.tile_pool(name="persist", bufs=1))

        # persistent SBUF tensors, indexed by batch b (the core owns the
        # same 2 heads in both batches).
        qts = [persist.tile([128, T], F32R, tag=f"qt{p}", name=f"qt{p}")
               for p in range(2)]
        # zero-padded per-head K^T (head h lives in rows 64*(h%2);
        # the other 64 rows are zero so scores run as full K=128 matmuls)
        kts = [persist.tile([128, T], F32R, tag=f"kt{h}", name=f"kt{h}")
               for h in range(4)]
        va = persist.tile([128, 2, NKK, 192], BF16, tag="va")  # [V_e|ones|V_o]
        wsb = persist.tile([128, NCT, 384], BF16, tag="wsb")
        bsb = persist.tile([128, 3], F32, tag="bsb")
        ident = persist.tile([128, 128], BF16, tag="ident")
        wosb = persist.tile([128, NCT, C], BF16, tag="wo")
        bosb = persist.tile([128, C], F32, tag="bo")
        # attention output re-sharded to this core's units, src-major
        # (contraction dim c = 128*src + head-dim)
        yts = [persist.tile([128, NCT, HTQ], BF16, tag=f"yts{p}",
                            name=f"yts{p}") for p in range(2)]
        scr = persist.tile([128, 1], F32, tag="scr")
        # 0/1 keep-mask for the 128x128 diagonal blocks (c >= r); one tile
        # serves every diagonal block of every chunk
        mskw = persist.tile([128, 128], BF16, tag="mskw")

        if trunc == "nil":
            # harness-floor probe: loop body = one memset + one out DMA
            with tc.tile_pool(name="nil", bufs=1) as npool:
                d = npool.tile([128, TQ], F32, tag="nil")
                nc.vector.memset(d[:], 0.0)
                nc.sync.dma_start(out[0:128, 0:TQ], d[:])
            return

        xpool = ctx.enter_context(tc.tile_pool(name="xtile", bufs=8))
        xsb = {}

        def emit_x_dma(tc0):
            # chunk 0: per-kc pieces so the very first matmul starts after
            # 128KB; later chunks: one whole-chunk DMA (8KB contiguous per
            # partition -- much better DMA line efficiency), alternating
            # between the SP and Pool queues
            xtile = xpool.tile([128, NCT, TQ], BF16, tag="x", name=f"x{tc0}")
            if tc0 == 0:
                # split pieces across both queues: 2x arrival rate for the
                # prefix-critical first chunk
                for kc in range(NCT):
                    q = nc.sync if kc % 2 == 0 else nc.gpsimd
                    q.dma_start(xtile[:, kc, :], xt[tc0, :, kc, :])
            elif tc0 % 2:
                nc.sync.dma_start(xtile[:], xt[tc0])
            else:
                nc.gpsimd.dma_start(xtile[:], xt[tc0])
            xsb[tc0] = xtile

        # weights on the Act DMA queue so they overlap the x stream (SP);
        # per-kc pieces so the first g_step matmul starts ~1us in
        for kc in range(NCT):
            nc.scalar.dma_start(wsb[:, kc, :],
                                wqkv[128 * kc:128 * (kc + 1), :])
        nc.scalar.dma_start(bsb[:], bqkv[:])
        nc.scalar.dma_start(ident[:], ident_d[:])
        nc.scalar.dma_start(mskw[:], mskw_d[:])
        # x stream — PE needs tc0=0 immediately; the rest trickles in
        # under the attention stream
        for tc0 in range(NTC):
            emit_x_dma(tc0)

        # constants via on-engine memsets (no DMA):
        # kts zero padding + the ones block of VA
        for h in range(4):
            dead = slice(64, 128) if h % 2 == 0 else slice(0, 64)
            nc.vector.memset(kts[h][dead, :].bitcast(F32), 0.0)
        nc.vector.memset(va[:, :, :, 64:128], 1.0)

        # warm the Act Exp table off the critical path
        nc.vector.memset(scr[:], 0.0)
        nc.scalar.activation(scr[:], scr[:], AF.Exp)

        vpool = ctx.enter_context(tc.tile_pool(name="vtile", bufs=2))
        ospool = ctx.enter_context(tc.tile_pool(name="osb", bufs=2))

        def proj_steps(tc0, pp, ptr):
            """Yield the projection of t-chunk tc0 as schedulable steps:
            3 g-steps (8 matmuls + bias add) + 2 transpose-steps."""
            b, jloc = tc0 // NJ, tc0 % NJ
            chunk = slice(TQ * jloc, TQ * (jloc + 1))
            vtile = [None]

            def g_step(g):
                gcol = 128 * g
                ps = pp.tile([128, TQ], F32, tag="pp", name=f"pp{tc0}_{g}")
                for kc in range(NCT):
                    nc.tensor.matmul(ps[:], wsb[:, kc, gcol:gcol + 128],
                                     xsb[tc0][:, kc, :],
                                     start=(kc == 0), stop=(kc == NCT - 1))
                if g == 0:        # Q^T of batch b
                    nc.vector.tensor_scalar_add(qts[b][:, chunk], ps[:],
                                                bsb[:, 0:1])
                elif g == 1:      # K^T of batch b, split per head
                    nc.vector.tensor_scalar_add(
                        kts[2 * b][0:64, chunk], ps[0:64, :], bsb[0:64, 1:2])
                    nc.vector.tensor_scalar_add(
                        kts[2 * b + 1][64:128, chunk], ps[64:128, :],
                        bsb[64:128, 1:2])
                else:             # V of batch b
                    vt = vpool.tile([128, TQ], BF16, tag="v", name=f"v{tc0}")
                    nc.vector.tensor_scalar_add(vt[:], ps[:], bsb[:, 2:3])
                    vtile[0] = vt

            def t_step(qpair):
                for q in (2 * qpair, 2 * qpair + 1):
                    tt = jloc * 4 + q   # k-tile index in batch b
                    pst = ptr.tile([128, 128], BF16, tag="pt",
                                   name=f"pt{tc0}_{q}")
                    nc.tensor.matmul(pst[:], vtile[0][:, 128 * q:128 * (q + 1)],
                                     ident[:], is_transpose=True)
                    nc.vector.tensor_copy(va[:, b, tt, 0:64], pst[:, 0:64])
                    nc.vector.tensor_copy(va[:, b, tt, 128:192],
                                          pst[:, 64:128])

            yield from (lambda g=g: g_step(g) for g in range(3))
            yield from (lambda qp=qp: t_step(qp) for qp in range(2))

        # ---- prefix: projection of t-chunk 0 only ----
        with (
            tc.tile_pool(name="pp_pre", bufs=3, space="PSUM") as pp_pre,
            tc.tile_pool(name="ptr_pre", bufs=2, space="PSUM") as ptr_pre,
        ):
            for step in proj_steps(0, pp_pre, ptr_pre):
                step()
            if trunc == "proj":
                for tc0 in range(1, 8):
                    for step in proj_steps(tc0, pp_pre, ptr_pre):
                        step()
        if trunc == "proj":
            with tc.tile_pool(name="dum", bufs=1) as dpool:
                d = dpool.tile([128, TQ], F32, tag="d")
                nc.vector.tensor_copy(d[:], qts[0][:, 0:TQ].bitcast(F32))
                nc.sync.dma_start(out[0:128, 0:TQ], d[:])
            return

        def emit_collective(p):
            if with_collective is True:
                nc.gpsimd.collective_compute(
                    "AllToAll", mybir.AluOpType.bypass,
                    replica_groups=[list(range(NCORES))],
                    ins=[a2a_in[p]], outs=[a2a_out[p]])
            elif with_collective is False:
                nc.sync.dma_start(a2a_out[p], a2a_in[p])
            # else (None): timing mode — caller aliases a2a_out to a2a_in

        def emit_yts_dma(p):
            # gpsimd queue: ordered behind the collective, blocks nothing
            for src in range(NCORES):
                nc.gpsimd.dma_start(yts[p][:, src, :], a2a_out[p, src, :, :])

        def emit_pos_group(p, tt, n, pool, tag="pp"):
            """One output-projection accumulation group: 128 out rows
            (unit-local rows 128*tt..) x 512 out cols (512*n..)."""
            ps = pool.tile([128, TQ], F32, tag=tag, name=f"pos{p}_{tt}_{n}")
            for cc in range(NCT):
                nc.tensor.matmul(ps[:], yts[p][:, cc, 128 * tt:128 * (tt + 1)],
                                 wosb[:, cc, TQ * n:TQ * (n + 1)],
                                 start=(cc == 0), stop=(cc == NCT - 1))
            osb = ospool.tile([128, TQ], F32, tag="osb",
                              name=f"osb{p}_{tt}_{n}")
            nc.vector.tensor_add(osb[:], ps[:], bosb[:, TQ * n:TQ * (n + 1)])
            nc.sync.dma_start(
                out[HTQ * p + 128 * tt:HTQ * p + 128 * (tt + 1),
                    TQ * n:TQ * (n + 1)], osb[:])

        # ---- attention (p = batch index), filler-interleaved ----
        with (
            tc.tile_pool(name="pp", bufs=1, space="PSUM") as pp,
            tc.tile_pool(name="ptr", bufs=1, space="PSUM") as ptr,
            tc.tile_pool(name="psc", bufs=4, space="PSUM") as spool,
            tc.tile_pool(name="po", bufs=2, space="PSUM") as opool,
            tc.tile_pool(name="ptp", bufs=6) as ptpool,
            tc.tile_pool(name="yt", bufs=3) as ytpool,
            tc.tile_pool(name="rt", bufs=3) as rtpool,
            tc.tile_pool(name="oe", bufs=4) as oepool,
        ):
            # filler: projection of t-chunks 1..7, interleaved into the
            # attention stream under the _FILL_HARD deadline schedule
            filler = []
            for tc0 in range(1, 8):
                filler.extend(proj_steps(tc0, pp, ptr))

            nfill = len(filler)
            fill_state = {"done": 0}
            po_t = {}

            def fill_until(n):
                n = min(n, nfill)
                while fill_state["done"] < n:
                    filler[fill_state["done"]]()
                    fill_state["done"] += 1

            def emit_scores(p, j, kk):
                """Score matmuls + exp for one k-tile. Diagonal tiles get
                their 128x128 diagonal block zeroed POST-exp by a DVE
                multiply with the 0/1 triangle mask (no PE mask matmul).
                Diagonal score matmuls start at column KT*o, but no
                narrower than 256 (fp32r drops to 1/4 rate below N=256).
                Scores use per-head single-bank PSUM tiles (bufs=4 -> two
                full steps of lookahead for the software pipeline)."""
                diag = kk >= 4 * j
                o = max(kk - 4 * j, 0)  # suffix offset (diag tiles)
                om = min(KT * o, TQ - 256)  # matmul column start
                dec = trunc == "dec"  # timing probe: no exp, no mask
                pt = None if dec else ptpool.tile([128, 2 * TQ], BF16,
                                                  tag="pt",
                                                  name=f"p{p}_{j}_{kk}")
                for h2 in range(2):
                    ps_s = spool.tile([128, TQ], F32, tag="s",
                                      name=f"s{p}_{j}_{kk}_{h2}")
                    nc.tensor.matmul(
                        ps_s[:, om:],
                        kts[2 * p + h2][:, KT * kk:KT * (kk + 1)],
                        qts[p][:, TQ * j + om:TQ * (j + 1)],
                        start=True, stop=True)
                    if dec:
                        continue
                    lo = TQ * h2 + KT * o
                    nc.scalar.activation(pt[:, lo:TQ * (h2 + 1)],
                                         ps_s[:, KT * o:], AF.Exp)
                    if diag:
                        nc.vector.tensor_mul(pt[:, lo:lo + KT],
                                             pt[:, lo:lo + KT], mskw[:])
                return pt

            def emit_av(p, j, kk, pt):
                """Accumulate one k-tile into the (p, j) output."""
                nkk = 4 * (j + 1)
                if kk == 0:
                    po_t[(p, j)] = [
                        opool.tile([128, TQ], F32, tag="po",
                                   name=f"po{p}_{j}_{h}") for h in range(2)]
                po = po_t[(p, j)]
                o = max(kk - 4 * j, 0)
                for h2 in range(2):
                    vs = slice(0, 128) if h2 == 0 else slice(64, 192)
                    if trunc == "dec":
                        # decoupled probe: PE reads a ready bf16 region
                        # instead of the exp output -- no cross-engine dep
                        rhs = qts[p].bitcast(BF16)[:, KT * o:TQ]
                    else:
                        rhs = pt[:, TQ * h2 + KT * o:TQ * (h2 + 1)]
                    nc.tensor.matmul(
                        po[h2][:, KT * o:TQ],
                        va[:, p, kk, vs],
                        rhs,
                        start=(kk == 0), stop=(kk == nkk - 1))

            def emit_norm(p, j):
                """Normalize the finished (p, j) chunk and ship it.
                h0 sums sit in rows 64:128, h1 sums in rows 0:64."""
                po = po_t.pop((p, j))
                yt = ytpool.tile([128, TQ], BF16, tag="yt", name=f"y{p}_{j}")
                rt = rtpool.tile([128, TQ], F32, tag="rt", name=f"r{p}_{j}")
                # copy psum->sbuf fast so the accumulator banks free
                # for the next q-chunk before the recip/mul run
                oes = [oepool.tile([128, TQ], F32, tag="oe",
                                   name=f"oe{p}_{j}_{h}") for h in range(2)]
                nc.vector.tensor_copy(oes[0][:], po[0][:])
                nc.vector.tensor_copy(oes[1][:], po[1][:])
                nc.vector.reciprocal(rt[0:64, :], oes[0][64:128, :])
                nc.vector.tensor_mul(yt[0:64, :], oes[0][0:64, :], rt[0:64, :])
                nc.vector.reciprocal(rt[64:128, :], oes[1][0:64, :])
                nc.vector.tensor_mul(yt[64:128, :], oes[1][64:128, :],
                                     rt[64:128, :])
                for hv in range(2):
                    nc.sync.dma_start(a2a_in[p, 2 * j + hv, :, :],
                                      yt[:, HTQ * hv:HTQ * (hv + 1)])

            # flattened (p, j, kk) stream, software-pipelined with
            # lookahead 2: AV(k-2) is emitted after S(k)/exp(k) so the PE
            # queue never sits on the exp latency; projection filler rides
            # between S(k) and AV(k-2) under the deadline schedule.
            steps = [(p, j, kk) for p in range(2) for j in range(NJ)
                     for kk in range(4 * (j + 1))]
            # batch-0 output projection paced into late batch-1 attention
            pos_slots = {(1, 3, 10): (0, 0), (1, 3, 13): (0, 1)}
            pending = []

            def retire_one():
                pp_, jp_, kkp_, pt_ = pending.pop(0)
                emit_av(pp_, jp_, kkp_, pt_)
                if kkp_ == 4 * (jp_ + 1) - 1:
                    emit_norm(pp_, jp_)
                    if (pp_, jp_) == (0, NJ - 1):
                        # batch-0 done: launch its A2A under batch-1
                        emit_collective(0)
                        emit_yts_dma(0)

            for s, (p, j, kk) in enumerate(steps):
                fill_until(_fill_req(s))
                pt = emit_scores(p, j, kk)
                if len(pending) >= 2:
                    retire_one()
                if p == 0 and j == 0 and kk == 2:
                    # w_out prefetch on the Act queue, issued once the
                    # startup DMA burst has drained
                    nc.scalar.dma_start(
                        wosb[:], wout[:].rearrange("(n p) c -> p n c", p=128))
                    nc.scalar.dma_start(bosb[:], bout[:])
                if (p, j, kk) in pos_slots:
                    tt, n = pos_slots[(p, j, kk)]
                    emit_pos_group(0, tt, n, pp)
                pending.append((p, j, kk, pt))
            while pending:
                retire_one()
            emit_collective(1)
            emit_yts_dma(1)
            fill_until(nfill)   # safety: shouldn't trigger

        if trunc == "attn":
            with tc.tile_pool(name="dum2", bufs=1) as dpool2:
                d2 = dpool2.tile([128, TQ], F32, tag="d2")
                nc.vector.tensor_copy(d2[:], qts[0][:, 0:TQ].bitcast(F32))
                nc.sync.dma_start(out[0:128, 0:TQ], d2[:])
            return

        # ---- tail: remaining batch-0 groups (PE-warm while A2A#1 flies),
        # then batch-1's output projection ----
        poutp = ctx.enter_context(
            tc.tile_pool(name="pout", bufs=4, space="PSUM"))
        for tt, n in ((1, 0), (1, 1)):
            emit_pos_group(0, tt, n, poutp, tag="pout")
        for tt in range(2):
            for n in range(2):
                emit_pos_group(1, tt, n, poutp, tag="pout")


def make_core_inputs(x, w_qkv, b_qkv, w_out, b_out):
    """Host-side shard/transform. Returns list of per-core input dicts."""
    import ml_dtypes
    bf16 = ml_dtypes.bfloat16

    x = np.asarray(x, np.float32)
    w_qkv = np.asarray(w_qkv, np.float32)
    b_qkv = np.asarray(b_qkv, np.float32)
    w_out = np.asarray(w_out, np.float32)
    b_out = np.asarray(b_out, np.float32)

    bout_rep = np.ascontiguousarray(np.broadcast_to(b_out, (128, C)))
    # x^T tiles: xt[tc0, p, kc, q] = x_flat[TQ*tc0+q, 128*kc+p]
    xt = np.ascontiguousarray(
        x.reshape(NTC, TQ, NCT, 128).transpose(0, 3, 2, 1)).astype(bf16)
    wout_bf = w_out.astype(bf16)
    in_maps = []
    for c in range(NCORES):
        s = slice(128 * c, 128 * (c + 1))
        wq = w_qkv[:, :C][:, s] * 0.125
        wk = w_qkv[:, C:2 * C][:, s]
        wv = w_qkv[:, 2 * C:][:, s]
        wc = np.ascontiguousarray(
            np.concatenate([wq, wk, wv], axis=1)).astype(bf16)
        bc3 = np.ascontiguousarray(
            np.stack([b_qkv[:C][s] * 0.125, b_qkv[C:2 * C][s],
                      b_qkv[2 * C:][s]], axis=1))
        in_maps.append({
            "xt": xt, "wqkv": wc, "bqkv": bc3,
            "wout": wout_bf, "bout": bout_rep,
            "ident": np.eye(128, dtype=np.float32).astype(bf16),
            # keep q-col c of k-row r iff c >= r (strict upper + diag)
            "mskw": np.triu(np.ones((128, 128), np.float32)).astype(bf16),
        })
    return in_maps


_NC_CACHE = {}


def _make_cached_runner(nc):
    """Jit the SPMD executable once; subsequent calls only re-upload inputs."""
    import jax
    from jax.sharding import Mesh, PartitionSpec
    from jax.experimental.shard_map import shard_map
    from concourse.bass2jax import (_bass_exec_p, install_neuronx_cc_hook,
                                    partition_id_tensor)

    install_neuronx_cc_hook()
    partition_name = (nc.partition_id_tensor.name
                      if nc.partition_id_tensor else None)
    in_names, out_names, out_avals = [], [], []
    for alloc in nc.m.functions[0].allocations:
        if not isinstance(alloc, mybir.MemoryLocationSet):
            continue
        name = alloc.memorylocations[0].name
        if alloc.kind == "ExternalInput":
            if name != partition_name:
                in_names.append(name)
        elif alloc.kind == "ExternalOutput":
            out_names.append(name)
            out_avals.append(jax.core.ShapedArray(
                tuple(alloc.tensor_shape), mybir.dt.np(alloc.dtype)))
    n_params = len(in_names)
    all_in = list(in_names) + list(out_names)
    if partition_name is not None:
        all_in.append(partition_name)

    def _body(*args):
        operands = list(args)
        if partition_name is not None:
            operands.append(partition_id_tensor())
        outs = _bass_exec_p.bind(
            *operands, out_avals=tuple(out_avals), in_names=tuple(all_in),
            out_names=tuple(out_names), lowering_input_output_aliases=(),
            sim_require_finite=True, sim_require_nnan=True, nc=nc)
        return tuple(outs)

    devices = jax.devices()[:NCORES]
    mesh = Mesh(np.asarray(devices), ("core",))
    spec = PartitionSpec("core")
    sharded = jax.jit(
        shard_map(_body, mesh=mesh,
                  in_specs=(spec,) * (n_params + len(out_names)),
                  out_specs=(spec,) * len(out_names), check_rep=False),
        keep_unused=True)
    zeros = [np.zeros((NCORES * a.shape[0], *a.shape[1:]), a.dtype)
             for a in out_avals]

    def run(in_maps):
        concat = [np.concatenate([np.asarray(m[nm]) for m in in_maps], axis=0)
                  for nm in in_names]
        outs = sharded(*concat, *zeros)
        return {nm: np.asarray(outs[i]) for i, nm in enumerate(out_names)}

    return run


def _gather(per_core):
    """per_core: [NCORES*TSL, C] stacked core outputs -> [B, T, C].

    Core c rows 0:256 = batch-0 rows 256c..; rows 256:512 = batch-1."""
    A = per_core.reshape(NCORES, TSL, C)
    full = np.empty((BT, C), per_core.dtype)
    full[:T] = A[:, :HTQ].reshape(T, C)
    full[T:] = A[:, HTQ:].reshape(T, C)
    return full.reshape(B, T, C)


def kernel(x, w_qkv, b_qkv, w_out, b_out):
    in_maps = make_core_inputs(x, w_qkv, b_qkv, w_out, b_out)
    if "nc" not in _NC_CACHE:
        _NC_CACHE["nc"] = build()
    nc = _NC_CACHE["nc"]
    try:
        if "run" not in _NC_CACHE:
            _NC_CACHE["run"] = _make_cached_runner(nc)
        outs = _NC_CACHE["run"](in_maps)
        full = outs["out"].reshape(NCORES * TSL, C)
    except Exception:
        res = run_bass_kernel_spmd(nc, in_maps, core_ids=list(range(NCORES)))
        full = np.concatenate([res.results[c]["out"] for c in range(NCORES)],
                              axis=0)
    return _gather(full)
